# revision 10
# baseline (speedup 1.0000x reference)
"""MoE location-expert router kernel for Trainium2 (8 NeuronCores).

Problem: out[i] = W[ptr[i] % 8] @ x[i] + b[ptr[i] % 8]
  x [4096, 1024] f32, W [8, 32000, 1024] f32, b [8, 32000] f32 (zeros)
  out [4096, 32000] f32

Sharding: vocab / tensor-parallel. The host sorts tokens by expert and
each of the 8 cores owns a 4000-wide slice of the vocab dim of ALL
experts -> identical SPMD program per core, perfectly load balanced
for any routing distribution. Inputs are cast to fp16 (rel err ~3e-4,
tolerance 2e-2): halves weight DMA and enables the fast PE paths.

Device program (per core), built from trace evidence on HW:
  - Per 128-token tile: contraction chunk kc OUTER, vocab tile INNER,
    accumulating all 8 vocab tiles in the 8 PSUM banks at once; eight
    consecutive matmuls share one stationary x-tile.
  - Full per-expert weight slice [128, 8kc, 4000] fp16 (64KB/partition)
    resident in SBUF, double buffered; the next expert's slices are
    prefetched one kc per token tile (no DMA burst), on the sync ring.
  - Remainder tokens (count % 128) are padded to 32 and packed into
    col-tiled passes: up to four 32-token groups (from the two resident
    experts) run CONCURRENTLY in the PE array via tile_position
    col-groups, sharing the 8 PSUM banks. Banks are zeroed by the
    scalar engine and the matmuls run start=False (accumulate onto
    zeros) because a start=True bank-wide has_written clear races the
    other col-groups' writes. Saves 2 of 8 remainder tile passes.
  - PSUM banks drain (f32 -> fp16 cast) on the vector engine into
    half-vocab staging tiles; out DMAs go on the scalar HWDGE ring
    ONLY (the sync ring carries loads only) -- out DMAs queuing behind
    weight prefetch was the main PE-stall mechanism (casts block on
    staging-slot WAR, PE blocks on PSUM WAR).
  - x token tiles are issued one expert ahead of their weights; expert
    0's tokens ride the scalar ring so xe0 and W0[kc0] fill during the
    Tile preamble concurrently.
  - Experts are processed in descending-remainder order so the final
    out DMA (kernel tail) covers only a few rows.

Host: routes/sorts tokens, pads, transposes; scatters core outputs
back to token order. b is zero in this problem (added on host if not).
"""

import os

import numpy as np

import concourse.bacc as bacc
import concourse.bass as bass
import concourse.mybir as mybir
import concourse.tile as tile
from concourse.bass_utils import run_bass_kernel_spmd

E = 8          # experts
D = 1024       # d_model
V = 32000      # vocab
B = 4096       # tokens
NCORES = 8
VS = V // NCORES       # vocab slice per core (4000)
KT = 128               # contraction tile (partition dim)
KC = D // KT           # 8 K-chunks
MT = 128               # token tile (PSUM partition dim)
NT = 500               # vocab tile (moving free dim, <=512 for one PSUM bank)
NV = VS // NT          # 8 vocab tiles per core

MODE = os.environ.get("KERNEL_MODE", "fp16")

_program_cache = {}


def _build_program(pad_counts, counts, mode):
    """Trace the SPMD Tile program for the given per-expert padded counts."""
    if mode == "fp16":
        io_dt = mybir.dt.float16
    elif mode == "bf16":
        io_dt = mybir.dt.bfloat16
    else:
        io_dt = mybir.dt.float32r

    m_total = int(sum(pad_counts))
    nc = bacc.Bacc("TRN2", target_bir_lowering=False, debug=False,
                   enable_asserts=False, num_devices=NCORES)

    xT = nc.dram_tensor("xT", [D, m_total], io_dt, kind="ExternalInput").ap()
    wT = nc.dram_tensor("wT", [E, D, VS], io_dt, kind="ExternalInput").ap()
    out_dt = mybir.dt.float16 if mode == "fp16" else mybir.dt.float32
    out = nc.dram_tensor("out", [B, VS], out_dt, kind="ExternalOutput").ap()

    xT_r = xT.rearrange("(kc p) m -> p kc m", p=KT)
    live_experts = [e for e in range(E) if int(pad_counts[e]) > 0]
    # process experts in descending-remainder order: the final tile's
    # out DMA (and so the kernel tail) covers only the smallest
    # remainder's rows
    live_experts.sort(key=lambda e: -(int(counts[e]) % MT or MT))

    with tile.TileContext(nc) as tc:
        with (
            tc.tile_pool(name="xp", bufs=2) as xpool,
            tc.tile_pool(name="wp", bufs=2) as wpool,
            tc.tile_pool(name="op", bufs=8) as opool,
            tc.tile_pool(name="ps", bufs=8, space="PSUM") as pspool,
        ):
            we_tiles = {}

            def ensure_we(ei):
                # ei indexes live_experts
                if ei >= len(live_experts):
                    return None, None
                if ei not in we_tiles:
                    e = live_experts[ei]
                    wt = wpool.tile([KT, KC, VS], io_dt, tag="w",
                                    name=f"we{e}")
                    we_tiles[ei] = (wt, wT[e].rearrange("(kc p) v -> p kc v",
                                                        p=KT))
                return we_tiles[ei]

            pad_offs = {}
            val_offs = {}
            po = vo = 0
            for e in range(E):
                pad_offs[e], val_offs[e] = po, vo
                po += int(pad_counts[e])
                vo += int(counts[e])

            xe_tiles = {}

            def ensure_xe(ei):
                if ei >= len(live_experts):
                    return None
                if ei not in xe_tiles:
                    e = live_experts[ei]
                    pe = int(pad_counts[e])
                    xt = xpool.tile([KT, KC, pe], io_dt, tag="x",
                                    name=f"xe{e}")
                    # expert 0's tokens ride the scalar ring so W0[kc0]
                    # (sync) and xe0 transfer concurrently at the head
                    eng = nc.scalar if ei == 0 else nc.sync
                    eng.dma_start(
                        out=xt[:, :, :],
                        in_=xT_r[:, :, pad_offs[e]:pad_offs[e] + pe],
                    )
                    xe_tiles[ei] = xt
                return xe_tiles[ei]

            # first expert's tokens BEFORE its weights: the first matmul
            # needs xe0 + W0[kc0]; everything else pipelines behind
            ensure_xe(0)
            we0, wT0 = ensure_we(0)
            for kc in range(KC):
                eng = nc.sync if kc % 2 == 0 else nc.scalar
                eng.dma_start(out=we0[:, kc, :], in_=wT0[:, kc, :])

            # remainder 32-granule chunks per expert:
            # (col offset in xe, valid rows, out row0)
            chunks = {}
            for ei, e in enumerate(live_experts):
                c, r = int(counts[e]), int(counts[e]) % MT
                full = c // MT
                chunks[ei] = [
                    (full * MT + 32 * j, min(32, r - 32 * j),
                     val_offs[e] + full * MT + 32 * j)
                    for j in range(-(-r // 32))
                ]

            # greedy boundary packing: at boundary (ei, ei+1) consume all
            # of ei's leftover chunks, topping up passes with ei+1's
            passes_at = {i: [] for i in range(len(live_experts))}
            left = {ei: list(chunks[ei]) for ei in chunks}
            nb = len(live_experts) - 1
            for i in range(nb):
                must = [(i, ch) for ch in left[i]]
                left[i] = []
                if i == nb - 1:
                    must += [(i + 1, ch) for ch in left[i + 1]]
                    left[i + 1] = []
                elif must:
                    cap = -(-len(must) // 4) * 4 - len(must)
                    take, left[i + 1] = left[i + 1][:cap], left[i + 1][cap:]
                    must += [(i + 1, ch) for ch in take]
                for j in range(0, len(must), 4):
                    passes_at[i + 1].append(must[j:j + 4])

            def packed_pass(groups):
                """One 128-col pass: up to 4 col-tiled 32-token groups,
                possibly from different (resident) experts, sharing the 8
                PSUM banks. Group 0's kc0 matmuls run start=True FIRST
                (serialized on its col-group) so the bank-wide has_written
                clear lands before any other group's writes."""
                psts = [pspool.tile([MT, NT], mybir.dt.float32, tag="ps",
                                    name=f"pp{v}")
                        for v in range(NV)]
                # zero the banks and run every matmul with start=False:
                # first write per element overwrites the zero (bit clear
                # from prior start-group) or accumulates onto zero -- both
                # correct, and no bank-wide has_written clear can race the
                # other col-groups' writes
                for v in range(NV):
                    nc.scalar.mul(out=psts[v][:, :], in_=psts[v][:, :],
                                  mul=0.0)
                for kc in range(KC):
                    for v in range(NV):
                        for g, (gei, (coff, gval, grow0)) in enumerate(groups):
                            xe_g = xe_tiles[gei]
                            we_g = we_tiles[gei][0]
                            lhsT = xe_g[:, kc, coff:coff + 32]
                            nc.tensor.matmul(
                                psts[v][32 * g:32 * g + 32, :], lhsT,
                                we_g[:, kc, v * NT:(v + 1) * NT],
                                start=False, stop=(kc == KC - 1),
                                tile_position=(0, 32 * g),
                                skip_group_check=True,
                            )
                for h in range(2):
                    oth = opool.tile([MT, VS // 2], out_dt, tag="o",
                                     name=f"otp{h}")
                    for v in range(NV // 2):
                        vv = h * (NV // 2) + v
                        nc.vector.tensor_copy(
                            oth[:, v * NT:(v + 1) * NT], psts[vv][:, :])
                    for g, (gei, (coff, gval, grow0)) in enumerate(groups):
                        nc.scalar.dma_start(
                            out=out[grow0:grow0 + gval,
                                    h * (VS // 2):(h + 1) * (VS // 2)],
                            in_=oth[32 * g:32 * g + gval, :],
                        )

            for ei, e in enumerate(live_experts):
                pe = int(pad_counts[e])
                pad_off, val_off = pad_offs[e], val_offs[e]
                xe = ensure_xe(ei)
                ensure_xe(ei + 1)   # next expert's tokens, ahead of its W
                we, _ = ensure_we(ei)
                nxt_we, nxt_wT = ensure_we(ei + 1)
                ntiles = int(counts[e]) // MT   # full tiles only
                per = -(-KC // max(1, ntiles))
                kc_next = 0
                pending = list(passes_at[ei])
                for t in range(ntiles):
                    # spread next expert's weight prefetch across tiles
                    if nxt_we is not None:
                        for _ in range(per):
                            if kc_next < KC:
                                nc.sync.dma_start(
                                    out=nxt_we[:, kc_next, :],
                                    in_=nxt_wT[:, kc_next, :],
                                )
                                kc_next += 1
                    psts = [pspool.tile([MT, NT], mybir.dt.float32, tag="ps",
                                        name=f"ps{v}")
                            for v in range(NV)]
                    for kc in range(KC):
                        lhsT = xe[:, kc, t * MT:(t + 1) * MT]
                        for v in range(NV):
                            nc.tensor.matmul(
                                psts[v][:, :], lhsT,
                                we[:, kc, v * NT:(v + 1) * NT],
                                start=(kc == 0), stop=(kc == KC - 1),
                            )
                    valid = min(MT, int(counts[e]) - t * MT)
                    r0 = val_off + t * MT
                    # half-vocab staging tiles (4KB/partition, bufs=8):
                    # doubles the cast->out-DMA WAR slack and halves each
                    # HBM write burst. Out DMAs stay off the sync ring.
                    for h in range(2):
                        oth = opool.tile([MT, VS // 2], out_dt, tag="o",
                                         name=f"ot{h}")
                        for v in range(NV // 2):
                            vv = h * (NV // 2) + v
                            # drains split across engines: banks 0-3 on
                            # DVE, 4-7 on ACT (parallel PSUM access on
                            # disjoint banks) so the next tile's matmuls
                            # unblock sooner
                            if h == 0:
                                nc.vector.tensor_copy(
                                    oth[:, v * NT:(v + 1) * NT],
                                    psts[vv][:, :])
                            else:
                                nc.scalar.copy(
                                    out=oth[:, v * NT:(v + 1) * NT],
                                    in_=psts[vv][:, :])
                        nc.scalar.dma_start(
                            out=out[r0:r0 + valid,
                                    h * (VS // 2):(h + 1) * (VS // 2)],
                            in_=oth[:valid, :],
                        )
                    # packed remainder passes for boundary (ei-1, ei): run
                    # after the first full tiles, while both experts' W are
                    # resident
                    if pending:
                        packed_pass(pending.pop(0))
                # flush: experts with few/zero full tiles must still issue
                # the next expert's remaining weight prefetch and passes
                if nxt_we is not None:
                    while kc_next < KC:
                        nc.sync.dma_start(
                            out=nxt_we[:, kc_next, :],
                            in_=nxt_wT[:, kc_next, :],
                        )
                        kc_next += 1
                while pending:
                    packed_pass(pending.pop(0))
    nc.compile()
    return nc, m_total


def _get_program(counts, mode):
    # full 128-tiles plus remainder padded to 32 (packed passes)
    pad_counts = tuple(int((c // MT) * MT + -(-(c % MT) // 32) * 32)
                       for c in counts)
    key = (pad_counts, tuple(int(c) for c in counts), mode)
    if key not in _program_cache:
        _program_cache[key] = _build_program(pad_counts, counts, mode)
    return pad_counts, _program_cache[key]


def _prepare(x, pointer_addresses, W, mode):
    idx = (np.asarray(pointer_addresses).astype(np.int64) % E).astype(np.int32)
    counts = np.bincount(idx, minlength=E)
    order = np.argsort(idx, kind="stable")
    pad_counts, (nc, m_total) = _get_program(tuple(counts), mode)

    np_dt = np.dtype("float32")
    if mode == "fp16":
        np_dt = np.dtype(np.float16)
    elif mode == "bf16":
        import ml_dtypes
        np_dt = np.dtype(ml_dtypes.bfloat16)

    x = np.asarray(x, dtype=np.float32)
    xs = x[order]                      # [B, D] sorted by expert
    x_pad = np.zeros((m_total, D), dtype=np_dt)
    row = 0
    srow = 0
    for e in range(E):
        c = int(counts[e])
        x_pad[row:row + c] = xs[srow:srow + c]
        row += int(pad_counts[e])
        srow += c
    xT = np.ascontiguousarray(x_pad.T)  # [D, m_total]

    W = np.asarray(W)
    wts = []
    for c in range(NCORES):
        Wc = W[:, c * VS:(c + 1) * VS, :]                 # [E, VS, D] view
        WTc = np.ascontiguousarray(Wc.transpose(0, 2, 1))  # [E, D, VS]
        if mode in ("fp16", "bf16"):
            WTc = WTc.astype(np_dt)
        wts.append(WTc)
    return idx, order, nc, xT, wts


def _run(x, pointer_addresses, W, b, trace=False, mode=None):
    mode = mode or MODE
    idx, order, nc, xT, wts = _prepare(x, pointer_addresses, W, mode)
    in_maps = [{"xT": xT, "wT": wts[c]} for c in range(NCORES)]
    kw = {}
    if trace:
        kw = dict(trace=True, trace_cores=[0])
    res = run_bass_kernel_spmd(nc, in_maps, list(range(NCORES)), **kw)

    out = np.empty((B, V), dtype=np.float32)
    for c in range(NCORES):
        out[order, c * VS:(c + 1) * VS] = res.results[c]["out"]

    b = np.asarray(b)
    if b.any():
        for e in range(E):
            out[idx == e] += b[e].astype(np.float32)
    return out, res


def kernel(x, pointer_addresses, W, b):
    out, _ = _run(x, pointer_addresses, W, b, trace=False)
    return out
